# revision 43
# baseline (speedup 1.0000x reference)
"""Trainium2 Bass kernel for nn_DetectorHelper (seq2seq LSTM anomaly detector).

Architecture: encoder LSTM over T=1024 steps -> decoder LSTM over reversed
sequence emitting a linear projection of the hidden state before each cell
update. Data-parallel over the batch axis: 8 NeuronCores x 16 batch rows.

Per-core design (state-stationary matmuls, batch-major cell math, bf16
matmul datapath):
  - gates [16, 4H] accumulate in PSUM from 6 bf16 matmuls per step
    (x-sliver + two hT K-tiles, two 512-col chunks = exactly one PSUM bank
    each). Gate order is [i f o g] (host-side row permute): the [i f]
    chunk streams first so sigmoid(i,f) — one [16,512] ACT op — runs
    while the [o g] chunk's matmuls stream; tanh(g) and sigmoid(o) follow
    the second chunk. The cell tail (tanh(c) -> h -> transpose -> hT copy)
    is split into 128-col halves so the k0 half of the next stationary
    operand is ready ~400ns before the k1 half, letting the next step's
    first h-matmul start earlier.
  - bf16 weights/x/h run the decoder projection matmuls at 1 cycle/row
    (fp32r pays 4x below 256 moving cols) and halve SBUF weight traffic.
    End-to-end relative error ~2.5e-3 (the LSTM recurrence is contractive;
    the cell state c stays fp32).
  - cell update on ACT (sigmoid/tanh) + DVE, with f*c on the otherwise-idle
    GPSIMD; h is transposed back to [H, 16] via two PE transposes for the
    next step's stationary operand.
  - For_i blocks of U=64 steps; x staged 8 steps per DMA into a 16-slot
    ring (two ping-pong halves), prefetched one group ahead, with one
    GPSIMD bf16-cast per group instead of per step.

Rejected by measurement/hardware: per-step cross-core exchange (DMA fixed
cost ~1-2us x 2048 steps), PSUM-banded 8-step x-preload (engines cannot
shift partition base between operands; matmul PSUM outputs must be
32-aligned), emitting the next step's x-matmuls before the transposes
(open PSUM accumulation groups interleaved with transpose matmuls hang
the device), whole-block x staging (exposes the DMA at For_i entry).
"""

import sys

sys.path.insert(0, "/opt/trn_rl_repo")

from contextlib import ExitStack

import numpy as np

B = 16      # batch rows per core
F = 64      # feature dim
H = 256     # hidden dim
G = 4 * H   # gate dim
T = 1024
U = 64      # timesteps per For_i body
XS = 16     # x staging ring slots (two 8-step groups, ping-pong)
N_CORES = 8

_CACHE = {}


def _build(repeat=1, external_io=True):
    import concourse.bass as bass
    import concourse.tile as tile
    from concourse import bacc, mybir

    F32 = mybir.dt.float32
    BF16 = mybir.dt.bfloat16
    NB = T // U

    nc = bacc.Bacc("TRN2", target_bir_lowering=False, debug=False,
                   num_devices=N_CORES)

    KI = "ExternalInput" if external_io else "Internal"
    KO = "ExternalOutput" if external_io else "Internal"
    xte_d = nc.dram_tensor("xte", [F + 1, T * B], F32, kind=KI).ap()
    xtd_d = nc.dram_tensor("xtd", [F + 1, T * B], F32, kind=KI).ap()
    wih_e_d = nc.dram_tensor("wih_e", [F + 1, G], F32, kind=KI).ap()
    whh_e_d = nc.dram_tensor("whh_e", [128, 2 * G], F32, kind=KI).ap()
    wih_d_d = nc.dram_tensor("wih_d", [F + 1, G], F32, kind=KI).ap()
    whh_d_d = nc.dram_tensor("whh_d", [128, 2 * G], F32, kind=KI).ap()
    wout_d = nc.dram_tensor("wout", [128, 2 * F], F32, kind=KI).ap()
    bout_d = nc.dram_tensor("bout", [B, F], F32, kind=KI).ap()
    ident_d = nc.dram_tensor("ident", [B, B], F32, kind=KI).ap()
    out_d = nc.dram_tensor("out", [B, T * F], F32, kind=KO).ap()
    sink_d = None
    if not external_io:
        sink_d = nc.dram_tensor("sink", [1, 4], F32,
                                kind="ExternalOutput").ap()

    with tile.TileContext(nc) as tc, ExitStack() as ctx:
        wpool = ctx.enter_context(tc.tile_pool(name="wpool", bufs=1))
        wih_e = wpool.tile([F + 1, G], BF16, name="wih_e_sb")
        whh_e = wpool.tile([128, 2 * G], BF16, name="whh_e_sb")
        wih_d = wpool.tile([F + 1, G], BF16, name="wih_d_sb")
        whh_d = wpool.tile([128, 2 * G], BF16, name="whh_d_sb")
        wout = wpool.tile([128, 2 * F], BF16, name="wout_sb")
        bout = wpool.tile([B, F], F32, name="bout_sb")
        ident = wpool.tile([B, B], F32, name="ident_sb")
        nc.sync.dma_start(bout[:], bout_d[:])
        nc.sync.dma_start(ident[:], ident_d[:])
        # matmul operands are bf16: DMA fp32 to staging, cast-copy on DVE.
        for sb, dr in [(wih_e, wih_e_d), (whh_e, whh_e_d), (wih_d, wih_d_d),
                       (whh_d, whh_d_d), (wout, wout_d)]:
            stg = wpool.tile(list(sb.shape), F32, name="wstg", tag="wstg", bufs=2)
            nc.sync.dma_start(stg[:], dr[:])
            nc.vector.tensor_copy(sb[:], stg[:])

        # persistent state, parity ping-pong: step j reads half p=j%2, writes 1-p
        hT = wpool.tile([128, 64], BF16, name="hT_sb")
        cst = wpool.tile([B, 2 * H], F32, name="c_sb")
        zinit = wpool.tile([128, 32], F32, name="zinit_sb")

        def init_state():
            nc.vector.memset(zinit[:], 0.0)
            nc.vector.tensor_copy(hT[:, 0:32], zinit[:])
            nc.vector.memset(cst[:, 0:H], 0.0)

        xstage = wpool.tile([F + 1, XS * B], F32, name="xstage_sb")
        xstager = wpool.tile([F + 1, XS * B], BF16, name="xstager_sb")

        gpool = ctx.enter_context(tc.tile_pool(name="gpool", bufs=2, space="PSUM"))
        tpool = ctx.enter_context(tc.tile_pool(name="tpool", bufs=2, space="PSUM"))
        opool = ctx.enter_context(tc.tile_pool(name="opool", bufs=2, space="PSUM"))
        apool = ctx.enter_context(tc.tile_pool(name="apool", bufs=3))
        cpool = ctx.enter_context(tc.tile_pool(name="cpool", bufs=6))
        spool = ctx.enter_context(tc.tile_pool(name="spool", bufs=2))

        SIG = mybir.ActivationFunctionType.Sigmoid
        TANH = mybir.ActivationFunctionType.Tanh

        def stage_group(blk, g, xsrc_d):
            """DMA + bf16-cast the 8-step group g of block blk into the ring
            half it maps to (slots pipeline: group g uses slots 8(g%2)..+8)."""
            sl = B * ((8 * g) % XS)
            nc.sync.dma_start(xstage[:, sl:sl + 8 * B],
                              xsrc_d[:, bass.ts(blk * (U // 8) + g, 8 * B)])
            nc.gpsimd.tensor_copy(xstager[:, sl:sl + 8 * B],
                                  xstage[:, sl:sl + 8 * B])

        def step(blk, j, xsrc_d, wih, whh, dec_ostage=None):
            p = j % 2
            h_prev = hT[:, 32 * p:32 * p + 32]
            h_next = hT[:, 32 * (1 - p):32 * (1 - p) + 32]
            c_prev = cst[:, H * p:H * p + H]
            c_next = cst[:, H * (1 - p):H * (1 - p) + H]
            xslotr = xstager[:, B * (j % XS):B * (j % XS) + B]

            g_ps = gpool.tile([B, G], F32, name="g_ps")

            def chunk_mms(cs, cn):
                nc.tensor.matmul(g_ps[:, cs:cs + cn], xslotr, wih[:, cs:cs + cn],
                                 start=True, stop=False)
                nc.tensor.matmul(g_ps[:, cs:cs + cn], h_prev[:, 0:16],
                                 whh[:, cs:cs + cn], start=False, stop=False)
                nc.tensor.matmul(g_ps[:, cs:cs + cn], h_prev[:, 16:32],
                                 whh[:, G + cs:G + cs + cn], start=False, stop=True)

            # gate layout [i f o g]; [i f] chunk first so sigmoid(i,f)
            # overlaps the [o g] chunk's matmul streaming
            chunk_mms(0, 512)     # i, f
            chunk_mms(512, 512)   # o, g
            if j % 8 == 0 and j // 8 < U // 8 - 1:
                # prefetch the next 8-step group into the other ring half
                # (emitted after the matmuls so the GPSIMD cast queues behind
                # this step's f*c multiply, not ahead of it; 8 steps of slack
                # remain before the staged data is consumed)
                stage_group(blk, j // 8 + 1, xsrc_d)
            if dec_ostage is not None:
                # after the gate MMs so the in-order PE starts the
                # chain-critical h-matmuls first
                ostage, col = dec_ostage
                o_ps = opool.tile([B, F], F32, name="o_ps")
                nc.tensor.matmul(o_ps[:], h_prev[:, 0:16], wout[:, 0:F],
                                 start=True, stop=False)
                nc.tensor.matmul(o_ps[:], h_prev[:, 16:32], wout[:, F:2 * F],
                                 start=False, stop=True)
                nc.vector.tensor_add(ostage[:, col:col + F], o_ps[:], bout[:])

            # gate order [i f o g]: i=0:256, f=256:512, o=512:768, g=768:1024
            gact = apool.tile([B, G], F32, name="gact")
            nc.scalar.activation(gact[:, 0:512], g_ps[:, 0:512], SIG)
            fc = cpool.tile([B, H], F32, name="fc")
            nc.gpsimd.tensor_mul(fc[:], gact[:, H:2 * H], c_prev)
            nc.scalar.activation(gact[:, 768:1024], g_ps[:, 768:1024], TANH)
            nc.scalar.activation(gact[:, 512:768], g_ps[:, 512:768], SIG)
            ig = cpool.tile([B, H], F32, name="ig")
            nc.vector.tensor_mul(ig[:], gact[:, 0:H], gact[:, 3 * H:4 * H])
            nc.vector.tensor_add(c_next, ig[:], fc[:])
            # tail split in 128-col halves: tanh(c)->h->transpose->copy for
            # the k0 half completes early so the next step's first h-matmul
            # (stationary = h_next[:, 0:16]) starts ~400ns sooner
            tch = cpool.tile([B, H], F32, name="tch")
            h_bm = cpool.tile([B, H], F32, name="h_bm")
            t_ps = tpool.tile([128, 32], F32, name="t_ps", tag="tops")
            nc.scalar.activation(tch[:, 0:128], c_next[:, 0:128], TANH)
            nc.vector.tensor_mul(h_bm[:, 0:128], gact[:, 2 * H:2 * H + 128],
                                 tch[:, 0:128])
            nc.tensor.transpose(t_ps[:, 0:16], h_bm[:, 0:128], ident[:])
            nc.vector.tensor_copy(h_next[:, 0:16], t_ps[:, 0:16])
            nc.scalar.activation(tch[:, 128:256], c_next[:, 128:256], TANH)
            nc.vector.tensor_mul(h_bm[:, 128:256],
                                 gact[:, 2 * H + 128:3 * H], tch[:, 128:256])
            nc.tensor.transpose(t_ps[:, 16:32], h_bm[:, 128:256], ident[:])
            nc.vector.tensor_copy(h_next[:, 16:32], t_ps[:, 16:32])

        def body():
            init_state()
            with tc.For_i(0, NB) as blk:
                stage_group(blk, 0, xte_d)
                for j in range(U):
                    step(blk, j, xte_d, wih_e, whh_e)

            with tc.For_i(0, NB) as blk:
                stage_group(blk, 0, xtd_d)
                ostage = spool.tile([B, U * F], F32, name="ostage")
                for j in range(U):
                    # decoder step s emits the projection of h BEFORE the
                    # update; outputs land reversed within the block (col
                    # U-1-j), and the block is stored at t-range
                    # [T-(blk+1)U, T-blk*U)
                    step(blk, j, xtd_d, wih_d, whh_d,
                         dec_ostage=(ostage, (U - 1 - j) * F))
                nc.sync.dma_start(out_d[:, bass.ts((NB - 1) - blk, U * F)],
                                  ostage[:])

        if repeat == 1:
            body()
        else:
            with tc.For_i(0, repeat):
                body()
        if sink_d is not None:
            nc.sync.dma_start(sink_d[:], bout[0:1, 0:4])

    nc.compile()
    return nc


def host_prep(ts_batch, W_ih_enc, W_hh_enc, b_enc, W_ih_dec, W_hh_dec, b_dec,
              W_out, b_out):
    # permute gate rows from pytorch order [i f g o] to kernel order [i f o g]
    perm = np.concatenate([np.arange(0, 512), np.arange(768, 1024),
                           np.arange(512, 768)])

    def prep_w(W_ih, W_hh, b):
        W_ih = np.asarray(W_ih, np.float32)[perm]
        W_hh = np.asarray(W_hh, np.float32)[perm]
        b = np.asarray(b, np.float32)[perm]
        wihT = np.ascontiguousarray(W_ih.T)                              # [F, G]
        wih_aug = np.concatenate([wihT, b[None, :]], 0)
        whhT = W_hh.T                                                    # [H, G]
        whh_pack = np.concatenate([whhT[:128], whhT[128:]], 1)           # [128, 2G]
        return np.ascontiguousarray(wih_aug), np.ascontiguousarray(whh_pack)

    wih_e, whh_e = prep_w(W_ih_enc, W_hh_enc, b_enc)
    wih_d, whh_d = prep_w(W_ih_dec, W_hh_dec, b_dec)
    woutT = np.asarray(W_out, np.float32).T
    wout_pack = np.ascontiguousarray(np.concatenate([woutT[:128], woutT[128:]], 1))
    bout_b = np.ascontiguousarray(
        np.broadcast_to(np.asarray(b_out, np.float32)[None, :], (B, F)))
    ident = np.eye(B, dtype=np.float32)

    ts = np.asarray(ts_batch, np.float32)
    in_maps = []
    for d in range(N_CORES):
        tsl = ts[d * B:(d + 1) * B]                       # [16, T, F]
        xte = np.empty((F + 1, T * B), np.float32)
        xte[:F] = tsl.transpose(2, 1, 0).reshape(F, T * B)  # col = t*16 + b
        xte[F] = 1.0
        xtd = np.ascontiguousarray(
            xte.reshape(F + 1, T, B)[:, ::-1, :].reshape(F + 1, T * B))
        in_maps.append({
            "xte": np.ascontiguousarray(xte), "xtd": xtd,
            "wih_e": wih_e, "whh_e": whh_e,
            "wih_d": wih_d, "whh_d": whh_d,
            "wout": wout_pack, "bout": bout_b, "ident": ident,
        })
    return in_maps


def kernel(ts_batch, W_ih_enc, W_hh_enc, b_enc, W_ih_dec, W_hh_dec, b_dec,
           W_out, b_out):
    from concourse.bass_utils import run_bass_kernel_spmd

    if "nc" not in _CACHE:
        _CACHE["nc"] = _build()
    nc = _CACHE["nc"]

    in_maps = host_prep(ts_batch, W_ih_enc, W_hh_enc, b_enc, W_ih_dec,
                        W_hh_dec, b_dec, W_out, b_out)
    res = run_bass_kernel_spmd(nc, in_maps, core_ids=list(range(N_CORES)))
    outs = [r["out"].reshape(B, T, F) for r in res.results]
    return np.ascontiguousarray(np.concatenate(outs, 0))


if __name__ == "__main__":
    rng = np.random.default_rng(0)
    demo = {
        "ts_batch": rng.standard_normal((128, T, F), dtype=np.float32),
        "W_ih_enc": rng.standard_normal((G, F), dtype=np.float32) * 0.06,
        "W_hh_enc": rng.standard_normal((G, H), dtype=np.float32) * 0.06,
        "b_enc": rng.standard_normal(G).astype(np.float32) * 0.06,
        "W_ih_dec": rng.standard_normal((G, F), dtype=np.float32) * 0.06,
        "W_hh_dec": rng.standard_normal((G, H), dtype=np.float32) * 0.06,
        "b_dec": rng.standard_normal(G).astype(np.float32) * 0.06,
        "W_out": rng.standard_normal((F, H), dtype=np.float32) * 0.06,
        "b_out": rng.standard_normal(F).astype(np.float32) * 0.06,
    }
    out = kernel(**demo)
    print("kernel output", out.shape, out.dtype, float(np.abs(out).max()))
